# revision 27
# baseline (speedup 1.0000x reference)
"""CosyVoice2 attention (B=8, S=2048, H=896, 14Q/2KV GQA, RoPE, causal) as a
Trainium2 Bass/Tile kernel, data-parallel over batch across 8 NeuronCores.

v3: continuous-PE schedule (499us -> ~406us).  The TRN2 PE clock ramps
0.65->1.2->2.4GHz and only reaches 2.4GHz after ~3us of gap-free
execution; any idle resets it.  v2 ping-ponged PE<->ACT per attention
chunk, so most matmuls ran at 1.2-1.5GHz (467us PE busy for ~255us of
full-clock work).  v3 keeps the PE queue saturated: the middle of the
kernel measures 95-99% PE occupancy with matmuls at full clock (216ns
per 512-col stream).

Design:
  - attention backbone per (tile t, head-pair hp): for each 128-wide k-chunk
    kc: scores pair (two K=64 matmuls on opposite PE row-halves via
    tile_position - measured concurrent, the pair costs one 512-col stream)
    -> ACT exp (scale=1/8, bias=-4 folded; bf16 out) -> attnV pair (M=65
    with denominator ones-column) LAGGED two chunks behind the exp so the
    PE never waits on ACT.
  - psum: st [128,2,512] bufs=2 (4 banks) + av [65,2,512] bufs=1 (2) +
    proj [128,512] bufs=2 (2) = exactly 8 banks.  Proj groups are strictly
    sequential (single FIFO of fill generators) so 2 bufs ping-pong
    cleanly; rope's rotate-half matmul writes back into the projection
    psum tile (WAR tracked by the tile framework).
  - all projection / o_proj / rope work is sliced into single-matmul "fill
    units" interleaved between backbone steps at a slack-proportional rate
    (ACT per chunk ~2*csl/1.2GHz vs PE backbone 3*csl/2.4GHz), so PE slack
    is filled and the clock never drops.
  - normalize per pair: reciprocal_approx_fast on the [1,1024] denom row
    (psum->sbuf), gpsimd partition_broadcast to [64,1024], two DVE muls
    writing the bf16 A^T slab.
"""

import os
import sys

for _p in ("/opt/trn_rl_repo", "/root/.axon_site/_ro/trn_rl_repo"):
    if _p not in sys.path and os.path.isdir(_p):
        sys.path.append(_p)

import contextlib

import numpy as np
import ml_dtypes

import concourse.bacc as bacc
import concourse.mybir as mybir
import concourse.tile as tile
from concourse import bass_utils

B = 8
S = 2048
H = 896
NQ = 14
NKV = 2
D = 64
THETA = 1000000.0
P = 128
HC = H // P          # 7 hidden chunks
QT = 512             # q-tile width
NQT = S // QT        # 4 q-tiles
SC = S // P          # 16 seq chunks of 128
F32 = mybir.dt.float32
BF16 = mybir.dt.bfloat16
ADD = mybir.AluOpType.add
MULT = mybir.AluOpType.mult

_CACHE = {}
LAST_RESULTS = None

LAG = 3              # attnV trails exp by LAG chunks
UNIT_NS = 216.0      # one 512-col matmul at 2.4GHz


def _build():
    nc = bacc.Bacc("TRN2", target_bir_lowering=False, debug=False, num_devices=8)

    xt_d = nc.dram_tensor("xt", [P, HC, S], BF16, kind="ExternalInput").ap()
    wq_d = nc.dram_tensor("wq", [P, HC, H], BF16, kind="ExternalInput").ap()
    wk_d = nc.dram_tensor("wk", [P, HC, P], BF16, kind="ExternalInput").ap()
    wv_d = nc.dram_tensor("wv", [P, HC, P], BF16, kind="ExternalInput").ap()
    wo_d = nc.dram_tensor("wo", [P, HC, H], BF16, kind="ExternalInput").ap()
    bq_d = nc.dram_tensor("bqc", [P, HC], F32, kind="ExternalInput").ap()
    bk_d = nc.dram_tensor("bkc", [P, 1], F32, kind="ExternalInput").ap()
    bv_d = nc.dram_tensor("bvc", [P, 1], F32, kind="ExternalInput").ap()
    cos_d = nc.dram_tensor("cos4", [P, S], BF16, kind="ExternalInput").ap()
    sin_d = nc.dram_tensor("sinm4", [P, S], BF16, kind="ExternalInput").ap()
    pneg_d = nc.dram_tensor("pneg", [P, P], BF16, kind="ExternalInput").ap()
    ident_d = nc.dram_tensor("ident", [P, P], BF16, kind="ExternalInput").ap()
    o_d = nc.dram_tensor("o", [P, SC, H], F32, kind="ExternalOutput").ap()
    dbg = bool(int(os.environ.get("KERNEL_DEBUG", "0")))
    if dbg:
        kt_dbg = nc.dram_tensor("kt_dbg", [P, S], BF16,
                                kind="ExternalOutput").ap()
        as0_dbg = nc.dram_tensor("as0_dbg", [P, HC, QT], BF16,
                                 kind="ExternalOutput").ap()
        rcb_dbg = nc.dram_tensor("rcb_dbg", [D, 2, QT], F32,
                                 kind="ExternalOutput").ap()
        vp_dbg = nc.dram_tensor("vp_dbg", [P, SC, 130], BF16,
                                kind="ExternalOutput").ap()

    with tile.TileContext(nc) as tc, contextlib.ExitStack() as ctx:
        const = ctx.enter_context(tc.tile_pool(name="const", bufs=1))
        work = ctx.enter_context(tc.tile_pool(name="work", bufs=2))
        ppool = ctx.enter_context(tc.tile_pool(name="ppool", bufs=4))
        rpool = ctx.enter_context(tc.tile_pool(name="rpool", bufs=2))
        npool = ctx.enter_context(tc.tile_pool(name="npool", bufs=2))
        pst = ctx.enter_context(tc.tile_pool(name="pst", bufs=2, space="PSUM"))
        pav = ctx.enter_context(tc.tile_pool(name="pav", bufs=1, space="PSUM"))
        ppj = ctx.enter_context(tc.tile_pool(name="ppj", bufs=2, space="PSUM"))

        # ---- resident constants (order = DMA priority) ----
        wk_sb = const.tile([P, HC, P], BF16)
        wv_sb = const.tile([P, HC, P], BF16)
        bk_sb = const.tile([P, 1], F32)
        bv_sb = const.tile([P, 1], F32)
        bq_sb = const.tile([P, HC], F32)
        pneg_sb = const.tile([P, P], BF16)
        ident_sb = const.tile([P, P], BF16)
        wq_sb = const.tile([P, HC, H], BF16)
        wo_sb = const.tile([P, HC, H], BF16)
        bias_exp = const.tile([P, 1], F32)
        for dst, src in ((wk_sb, wk_d), (wv_sb, wv_d), (bk_sb, bk_d),
                         (bv_sb, bv_d), (bq_sb, bq_d), (pneg_sb, pneg_d),
                         (ident_sb, ident_d)):
            nc.sync.dma_start(out=dst, in_=src)
        nc.vector.memset(bias_exp, -4.0)

        # K^T resident and V' resident
        kt = const.tile([P, S], BF16)        # parts 0-63 = kv0, 64-127 = kv1
        vp = const.tile([P, SC, 130], BF16)  # [Vkv0 | ones | Vkv1 | ones]
        nc.vector.memset(vp[:, :, 64:65], 1.0)
        nc.vector.memset(vp[:, :, 129:130], 1.0)

        state = {}

        def gen_xdma(t):
            tslice = slice(t * QT, (t + 1) * QT)
            xs = work.tile([P, HC, QT], BF16, tag="xs", name=f"xs{t}")
            nc.sync.dma_start(out=xs, in_=xt_d[:, :, tslice])
            cos_t = work.tile([P, QT], BF16, tag="cos_t", name=f"cos{t}")
            sin_t = work.tile([P, QT], BF16, tag="sin_t", name=f"sin{t}")
            nc.sync.dma_start(out=cos_t, in_=cos_d[:, tslice])
            nc.sync.dma_start(out=sin_t, in_=sin_d[:, tslice])
            state[t] = {"xs": xs, "cos": cos_t, "sin": sin_t}

        # tile-0 inputs early, then the big weights
        gen_xdma(0)
        nc.sync.dma_start(out=wq_sb, in_=wq_d)
        nc.sync.dma_start(out=wo_sb, in_=wo_d)

        # absorb weight-DMA waits off the first real matmuls
        tch = ppj.tile([1, 2], F32, tag="proj", name="tch")

        def touch(t):
            ap = (t[0:1, 0, 0:2] if len(t.shape) == 3 else t[0:1, 0:2]).bitcast(F32)
            nc.tensor.matmul(tch[:, 0:1], ap, ap, start=True, stop=True)

        for t in (wk_sb, wv_sb, pneg_sb, ident_sb):
            touch(t)

        def rope_into(dst_ap, kp, bias_col, cos_t, sin_t, nm):
            """dst = (kp+b)*cos4 + Pneg @ ((kp+b)*sinm4).

            The rotate-half matmul reuses kp's psum tile (WAR on the two
            stt reads), so one proj psum buf covers the whole group.
            """
            t1 = rpool.tile([P, QT], F32, tag="t1", name=f"t1_{nm}")
            nc.vector.scalar_tensor_tensor(t1, kp, bias_col, cos_t,
                                           op0=ADD, op1=MULT)
            qe = rpool.tile([P, QT], BF16, tag="qe", name=f"qe_{nm}")
            nc.vector.scalar_tensor_tensor(qe, kp, bias_col, sin_t,
                                           op0=ADD, op1=MULT)
            yield
            nc.tensor.matmul(kp, pneg_sb, qe, start=True, stop=True)
            nc.vector.tensor_add(dst_ap, t1, kp)

        # ---------------- fill-unit generators ----------------
        def gen_kproj(t):
            st_t = state[t]
            tslice = slice(t * QT, (t + 1) * QT)
            kp = ppj.tile([P, QT], F32, tag="proj", name=f"kp{t}")
            for c in range(HC):
                nc.tensor.matmul(kp, wk_sb[:, c, :], st_t["xs"][:, c, :],
                                 start=(c == 0), stop=(c == HC - 1))
                yield
            yield from rope_into(kt[:, tslice], kp, bk_sb,
                                 st_t["cos"], st_t["sin"], f"k{t}")

        def gen_vproj(t):
            st_t = state[t]
            vtp = ppj.tile([P, QT], F32, tag="proj", name=f"vtp{t}")
            for c in range(HC):
                nc.tensor.matmul(vtp, wv_sb[:, c, :], st_t["xs"][:, c, :],
                                 start=(c == 0), stop=(c == HC - 1))
                yield
            vt_sb = rpool.tile([P, QT], BF16, tag="vt_sb", name=f"vt{t}")
            nc.vector.tensor_scalar_add(vt_sb, vtp, bv_sb)
            yield
            for j in range(4):
                sc_i = t * 4 + j
                vtr = ppj.tile([P, P], BF16, tag="proj", name=f"vtr{sc_i}")
                nc.tensor.transpose(vtr, vt_sb[:, j * P:(j + 1) * P], ident_sb)
                nc.vector.tensor_copy(vp[:, sc_i, 0:64], vtr[:, 0:64])
                nc.vector.tensor_copy(vp[:, sc_i, 65:129], vtr[:, 64:128])
                yield

        def gen_qproj(t, c):
            """Q projection chunk c (head-pair c) of tile t + rope."""
            st_t = state[t]
            qs = st_t["qs"]
            qp = ppj.tile([P, QT], F32, tag="proj", name=f"qp{t}_{c}")
            for hcc in range(HC):
                nc.tensor.matmul(qp, wq_sb[:, hcc, c * P:(c + 1) * P],
                                 st_t["xs"][:, hcc, :],
                                 start=(hcc == 0), stop=(hcc == HC - 1))
                yield
            yield from rope_into(qs[:, c, :], qp, bq_sb[:, c:c + 1],
                                 st_t["cos"], st_t["sin"], f"q{t}_{c}")

        parts = {}

        def gen_oproj_partial(j, n0, nw):
            """Last tile's o_proj: contract feat chunks 0..5 early (their
            aslab slabs are done before the final pair), park the partial
            sum in sbuf; the finisher adds chunk 6 after the last
            normalize.  Shrinks the serial epilogue tail."""
            t = NQT - 1
            aslab = state[t]["aslab"]
            sc_i = t * 4 + j
            jsl = slice(j * P, (j + 1) * P)
            op = ppj.tile([P, 512], F32, tag="proj", name=f"opp{sc_i}_{n0}")
            for c in range(HC - 1):
                nc.tensor.matmul(op[:, 0:nw], aslab[:, c, jsl],
                                 wo_sb[:, c, n0:n0 + nw],
                                 start=(c == 0), stop=(c == HC - 2))
                yield
            part = npool.tile([P, 512], F32, tag="opart", bufs=8,
                              name=f"part{sc_i}_{n0}")
            nc.vector.tensor_copy(part[:, 0:nw], op[:, 0:nw])
            parts[(j, n0)] = part
            yield

        def oproj_finish(j, n0, nw):
            t = NQT - 1
            aslab = state[t]["aslab"]
            sc_i = t * 4 + j
            jsl = slice(j * P, (j + 1) * P)
            op = ppj.tile([P, 512], F32, tag="proj", name=f"opf{sc_i}_{n0}")
            nc.tensor.matmul(op[:, 0:nw], aslab[:, 6, jsl],
                             wo_sb[:, 6, n0:n0 + nw], start=True, stop=True)
            osb = npool.tile([P, 512], F32, tag="osb", name=f"osf{sc_i}_{n0}")
            nc.vector.tensor_add(osb[:, 0:nw], parts.pop((j, n0))[:, 0:nw],
                                 op[:, 0:nw])
            nc.sync.dma_start(out=o_d[:, sc_i, n0:n0 + nw], in_=osb[:, 0:nw])

        def gen_oproj(t, j, n0, nw):
            """o_proj group: one 128-row seq chunk x one H-half."""
            aslab = state[t]["aslab"]
            sc_i = t * 4 + j
            jsl = slice(j * P, (j + 1) * P)
            op = ppj.tile([P, 512], F32, tag="proj", name=f"op{sc_i}_{n0}")
            for c in range(HC):
                nc.tensor.matmul(op[:, 0:nw], aslab[:, c, jsl],
                                 wo_sb[:, c, n0:n0 + nw],
                                 start=(c == 0), stop=(c == HC - 1))
                yield
            osb = npool.tile([P, 512], F32, tag="osb", name=f"os{sc_i}_{n0}")
            nc.vector.tensor_copy(osb[:, 0:nw], op[:, 0:nw])
            nc.sync.dma_start(out=o_d[:, sc_i, n0:n0 + nw], in_=osb[:, 0:nw])

        # ---------------- fill scheduler (strict FIFO) ----------------
        fills = []
        debt = 0.0

        def push(g):
            fills.append(g)

        def emit_dummy():
            # keep the PE streaming when no useful fill work remains: the
            # TRN2 PE clock drops to 1.2GHz after any idle and takes 3us of
            # continuous execution to return to 2.4GHz, so a dummy 512-col
            # matmul into the spare partitions of the av psum banks is
            # cheaper than an idle.
            d = state.get("avtile")
            if d is None:
                return
            nc.tensor.matmul(d[96:128, 0, 0:QT], wq_sb[:, 0, 0:32],
                             wq_sb[:, 1, 0:QT], start=True, stop=True,
                             tile_position=(0, 96), skip_group_check=True)

        def run_fill(n, pad=False):
            while n > 0 and fills:
                try:
                    next(fills[0])
                    n -= 1
                except StopIteration:
                    fills.pop(0)
            while pad and n > 0:
                emit_dummy()
                n -= 1
            return n

        def fill_for(slack_ns):
            nonlocal debt
            debt = min(debt + slack_ns / UNIT_NS, 8.0)
            k = int(debt)
            if k > 0:
                left = run_fill(k)
                debt -= k - left

        def drain(g):
            """Emit fills (FIFO) until generator g has fully completed.

            Engine queues execute in order, so a backbone instruction that
            depends on a fill unit DEADLOCKS unless that unit is emitted
            first.  drain() enforces the deadline-critical orderings.
            """
            if g is None:
                return
            if g in opq:
                while opq:
                    gg = opq.pop(0)
                    push(gg)
                    if gg is g:
                        break
            while g in fills:
                try:
                    next(fills[0])
                except StopIteration:
                    fills.pop(0)

        # ---------------- attention backbone ----------------
        def attention_pair(t, hp):
            qs = state[t]["qs"]
            aslab = state[t]["aslab"]
            nkc = (t + 1) * 4
            av = pav.tile([P, 2, QT], F32, tag="av", name=f"av{t}_{hp}")
            state["avtile"] = av
            probs = {}

            def emit_av(kc):
                c0 = (kc - 4 * t) * P if kc >= 4 * t else 0
                csl = slice(c0, QT)
                pr = probs.pop(kc)
                nc.tensor.matmul(av[0:65, 0, csl], vp[:, kc, 0:65],
                                 pr[:, 0, csl], start=(kc == 0),
                                 stop=(kc == nkc - 1), skip_group_check=True)
                nc.tensor.matmul(av[0:65, 1, csl], vp[:, kc, 65:130],
                                 pr[:, 1, csl], start=(kc == 0),
                                 stop=(kc == nkc - 1), skip_group_check=True)

            def emit_scores(kc):
                c0 = (kc - 4 * t) * P if kc >= 4 * t else 0
                csl = slice(c0, QT)
                ksl = slice(kc * P, (kc + 1) * P)
                st = pst.tile([P, 2, QT], F32, tag="st",
                              name=f"st{t}_{hp}_{kc}")
                nc.tensor.matmul(st[:, 0, csl], kt[0:64, ksl],
                                 qs[0:64, hp, csl], start=True, stop=True)
                nc.tensor.matmul(st[:, 1, csl], kt[64:128, ksl],
                                 qs[64:128, hp, csl], start=True, stop=True,
                                 tile_position=(64, 0))
                return st

            def emit_exp(kc, st):
                c0 = (kc - 4 * t) * P if kc >= 4 * t else 0
                csl = slice(c0, QT)
                pr = ppool.tile([P, 2, QT], BF16, tag="probs",
                                name=f"pr{t}_{hp}_{kc}")
                nc.scalar.activation(pr[:, :, csl], st[:, :, csl],
                                     mybir.ActivationFunctionType.Exp,
                                     bias=bias_exp, scale=0.125)
                if kc >= 4 * t:  # diagonal chunk: mask the triangle band
                    nc.gpsimd.affine_select(
                        out=pr[:, :, c0:c0 + P],
                        in_=pr[:, :, c0:c0 + P],
                        pattern=[[0, 2], [1, P]],
                        compare_op=mybir.AluOpType.is_ge, fill=0.0,
                        base=0, channel_multiplier=-1)
                probs[kc] = pr

            # chunks processed in twos: both score pairs back-to-back (the
            # second pair's LDWEIGHTS hides under the first pair's stream),
            # then both exps, then the lagged attnV pairs.
            for g in range(nkc // 2):
                k0, k1 = 2 * g, 2 * g + 1
                st0 = emit_scores(k0)
                st1 = emit_scores(k1)
                emit_exp(k0, st0)
                emit_exp(k1, st1)
                for kc in (k0, k1):
                    if kc >= LAG:
                        emit_av(kc - LAG)
                w2 = (QT - (k0 - 4 * t) * P if k0 >= 4 * t else QT) + \
                     (QT - (k1 - 4 * t) * P if k1 >= 4 * t else QT)
                slack = 1.1 * ((2 * w2 / 1.2 + 360.0) - (3 * w2 / 2.4 + 60.0))
                fill_for(slack)
            for kc in range(max(0, nkc - LAG), nkc):
                emit_av(kc)

            # evict av psum to sbuf FAST (frees the single av buf for the
            # next pair), then normalize lazily off-psum: recip ->
            # partition_broadcast -> muls (only gate o_proj, a tile later).
            nm = f"n{t}_{hp}"
            avs = npool.tile([D, 2, QT], BF16, tag="avs", name=f"avs{nm}")
            if t == 0 and hp == HC - 1:
                nc.scalar.copy(avs, av[0:64, :, :])
            else:
                nc.vector.tensor_copy(avs, av[0:64, :, :])
            dsb = npool.tile([1, 2, QT], F32, tag="dsb", name=f"dsb{nm}")
            if t == 0 or (t == NQT - 1 and hp == HC - 1):
                # early tiles are DVE-congested and ACT-idle (same for the
                # final pair): the slow single-partition denom copy goes to
                # the scalar engine
                nc.scalar.copy(dsb, av[64:65, :, :])
            else:
                nc.vector.tensor_copy(dsb, av[64:65, :, :])
            fill_for(1500.0 if t == 0 else 800.0)
            rc1 = npool.tile([1, 2, QT], F32, tag="rc1", name=f"rc1{nm}")
            nc.vector.reciprocal_approx_fast(rc1, dsb)
            rcb = npool.tile([D, 2, QT], F32, tag="rcb", name=f"rcb{nm}")
            nc.gpsimd.partition_broadcast(rcb, rc1)
            state["rcb_last"] = rcb
            nc.vector.tensor_mul(aslab[0:64, hp, :], avs[:, 0, :],
                                 rcb[:, 0, :])
            nc.vector.tensor_mul(aslab[64:128, hp, :], avs[:, 1, :],
                                 rcb[:, 1, :])

        # ---------------- schedule ----------------
        qgen = {}
        kvgen = {}
        ogen = {}
        opq = []
        # prologue: tile-0 K/V proj and first two Q chunks, run eagerly
        state[0]["qs"] = work.tile([P, HC, QT], BF16, tag="qs", name="qs0")
        state[0]["aslab"] = work.tile([P, HC, QT], BF16, tag="aslab",
                                      bufs=3, name="aslab0")
        for g in (gen_kproj(0), gen_vproj(0), gen_qproj(0, 0), gen_qproj(0, 1)):
            for _ in g:
                pass

        for t in range(NQT):
            qn = []
            if t == 0:
                for c in range(2, HC):
                    g = gen_qproj(0, c)
                    qgen[(0, c)] = g
                    qn.append(g)
            if t + 1 < NQT:
                gen_xdma(t + 1)
                state[t + 1]["qs"] = work.tile([P, HC, QT], BF16, tag="qs",
                                               name=f"qs{t + 1}")
                state[t + 1]["aslab"] = work.tile([P, HC, QT], BF16,
                                                  tag="aslab", bufs=3,
                                                  name=f"aslab{t + 1}")
                gk, gv = gen_kproj(t + 1), gen_vproj(t + 1)
                kvgen[t + 1] = [gk, gv]
                qn.append(gk)
                qn.append(gv)
                for c in range(HC):
                    g = gen_qproj(t + 1, c)
                    qgen[(t + 1, c)] = g
                    qn.append(g)
            if t >= 1:
                ogen[t - 1] = []
                for j in range(4):
                    for n0, nw in ((0, 512), (512, 384)):
                        g = gen_oproj(t - 1, j, n0, nw)
                        ogen[t - 1].append(g)
                        opq.append(g)
            # Q/K/V have a hard deadline (next tile start) and are pushed
            # now; o_proj groups (two tiles of slack via aslab bufs=3) are
            # admitted on demand at pair starts, reserving them for the
            # fill-starved late tiles.
            for g in qn:
                push(g)

            for hp in range(HC):
                # hard ordering requirements (deadlock avoidance):
                # qs chunk written before its scores; kt/vp tile-t section
                # written before this tile's diagonal chunks; o_proj(t-2)
                # done before this tile's normalize reuses its aslab buf.
                drain(qgen.get((t, hp)))
                if hp == 0:
                    for g in kvgen.get(t, ()):
                        drain(g)
                    for g in ogen.get(t - 3, ()):
                        drain(g)
                # lookahead: finish the NEXT pair's Q group now so its rope
                # (DVE) completes before that pair's first scores issue.
                nxt = (t, hp + 1) if hp + 1 < HC else (t + 1, 0)
                drain(qgen.get(nxt))
                if t == NQT - 1 and hp == HC - 1:
                    for j in range(4):
                        for n0, nw in ((0, 512), (512, 384)):
                            push(gen_oproj_partial(j, n0, nw))
                while opq and len(fills) <= (2 if t == NQT - 1 else 1):
                    push(opq.pop(0))
                attention_pair(t, hp)

        # epilogue: drain remaining fills, then last tile's o_proj
        for g in opq:
            push(g)
        opq.clear()
        run_fill(1 << 30)
        for j in range(4):
            for n0, nw in ((0, 512), (512, 384)):
                oproj_finish(j, n0, nw)
        if dbg:
            nc.sync.dma_start(out=kt_dbg, in_=kt)
            nc.sync.dma_start(out=as0_dbg, in_=state[0]["aslab"])
            nc.sync.dma_start(out=vp_dbg, in_=vp)
            nc.sync.dma_start(out=rcb_dbg, in_=state["rcb_last"])
    nc.compile()
    return nc


def _host_prep(hidden_states, position_ids, Wq, bq, Wk, bk, Wv, bv, Wo):
    """Build per-core input maps (host-side layout work)."""
    bf16 = ml_dtypes.bfloat16
    f32 = np.float32
    HALF = 32

    def chunked(w, dt=bf16):  # [H, N] -> [P, HC, N]
        return np.ascontiguousarray(
            w.reshape(HC, P, -1).transpose(1, 0, 2)).astype(dt)

    # q head-pair permutation: chunk p partitions 0-63 = head p (kv0),
    # 64-127 = head p+7 (kv1); Wo rows use the same ordering.
    perm = np.empty(H, np.int64)
    for hp in range(HC):
        for p in range(P):
            h = hp if p < 64 else hp + 7
            perm[hp * P + p] = h * D + (p % 64)
    wq_h = chunked(Wq[:, perm])
    wk_h = chunked(Wk)
    wv_h = chunked(Wv)
    wo_h = chunked(Wo[perm])
    bq_h = np.ascontiguousarray(bq[perm].reshape(HC, P).T).astype(f32)
    bk_h = bk.reshape(P, 1).astype(f32)
    bv_h = bv.reshape(P, 1).astype(f32)
    pneg = np.zeros((P, P), f32)
    for i in range(P):
        pneg[i, i ^ 32] = -1.0
    pneg_h = pneg.astype(bf16)
    ident_h = np.eye(P, dtype=f32).astype(bf16)

    inv_freq = (1.0 / (THETA ** (np.arange(0, HALF, dtype=np.float64) / HALF)))
    pidx = np.arange(P)
    sign = np.where((pidx % 64) >= HALF, 1.0, -1.0)[:, None]

    in_maps = []
    for b in range(B):
        xt = np.ascontiguousarray(
            hidden_states[b].T.reshape(HC, P, S).transpose(1, 0, 2)).astype(bf16)
        ang = position_ids[b].astype(np.float64)[None, :] * \
            inv_freq[pidx % HALF][:, None]          # [P, S]
        cos4 = np.cos(ang).astype(bf16)
        sinm4 = (np.sin(ang) * sign).astype(bf16)
        in_maps.append({
            "xt": xt, "wq": wq_h, "wk": wk_h, "wv": wv_h, "wo": wo_h,
            "bqc": bq_h, "bkc": bk_h, "bvc": bv_h,
            "cos4": cos4, "sinm4": sinm4, "pneg": pneg_h, "ident": ident_h,
        })
    return in_maps


def kernel(**inputs):
    global LAST_RESULTS
    if "nc" not in _CACHE:
        _CACHE["nc"] = _build()
    nc = _CACHE["nc"]
    in_maps = _host_prep(**inputs)
    trace = bool(int(os.environ.get("KERNEL_TRACE", "0")))
    res = bass_utils.run_bass_kernel_spmd(
        nc, in_maps, core_ids=list(range(8)), trace=trace)
    LAST_RESULTS = res
    out = np.empty((B, S, H), np.float32)
    for b in range(B):
        o = res.results[b]["o"]              # [P, SC, H]
        out[b] = o.transpose(1, 0, 2).reshape(S, H)
    return out


# revision 28
# speedup vs baseline: 1.0101x; 1.0101x over previous
"""CosyVoice2 attention (B=8, S=2048, H=896, 14Q/2KV GQA, RoPE, causal) as a
Trainium2 Bass/Tile kernel, data-parallel over batch across 8 NeuronCores.

v3: continuous-PE schedule (499us -> ~406us).  The TRN2 PE clock ramps
0.65->1.2->2.4GHz and only reaches 2.4GHz after ~3us of gap-free
execution; any idle resets it.  v2 ping-ponged PE<->ACT per attention
chunk, so most matmuls ran at 1.2-1.5GHz (467us PE busy for ~255us of
full-clock work).  v3 keeps the PE queue saturated: the middle of the
kernel measures 95-99% PE occupancy with matmuls at full clock (216ns
per 512-col stream).

Design:
  - attention backbone per (tile t, head-pair hp): for each 128-wide k-chunk
    kc: scores pair (two K=64 matmuls on opposite PE row-halves via
    tile_position - measured concurrent, the pair costs one 512-col stream)
    -> ACT exp (scale=1/8, bias=-4 folded; bf16 out) -> attnV pair (M=65
    with denominator ones-column) LAGGED two chunks behind the exp so the
    PE never waits on ACT.
  - psum: st [128,2,512] bufs=2 (4 banks) + av [65,2,512] bufs=1 (2) +
    proj [128,512] bufs=2 (2) = exactly 8 banks.  Proj groups are strictly
    sequential (single FIFO of fill generators) so 2 bufs ping-pong
    cleanly; rope's rotate-half matmul writes back into the projection
    psum tile (WAR tracked by the tile framework).
  - all projection / o_proj / rope work is sliced into single-matmul "fill
    units" interleaved between backbone steps at a slack-proportional rate
    (ACT per chunk ~2*csl/1.2GHz vs PE backbone 3*csl/2.4GHz), so PE slack
    is filled and the clock never drops.
  - normalize per pair: reciprocal_approx_fast on the [1,1024] denom row
    (psum->sbuf), gpsimd partition_broadcast to [64,1024], two DVE muls
    writing the bf16 A^T slab.
"""

import os
import sys

for _p in ("/opt/trn_rl_repo", "/root/.axon_site/_ro/trn_rl_repo"):
    if _p not in sys.path and os.path.isdir(_p):
        sys.path.append(_p)

import contextlib

import numpy as np
import ml_dtypes

import concourse.bacc as bacc
import concourse.mybir as mybir
import concourse.tile as tile
from concourse import bass_utils

B = 8
S = 2048
H = 896
NQ = 14
NKV = 2
D = 64
THETA = 1000000.0
P = 128
HC = H // P          # 7 hidden chunks
QT = 512             # q-tile width
NQT = S // QT        # 4 q-tiles
SC = S // P          # 16 seq chunks of 128
F32 = mybir.dt.float32
BF16 = mybir.dt.bfloat16
ADD = mybir.AluOpType.add
MULT = mybir.AluOpType.mult

_CACHE = {}
LAST_RESULTS = None

LAG = 3              # attnV trails exp by LAG chunks
UNIT_NS = 216.0      # one 512-col matmul at 2.4GHz


def _build():
    nc = bacc.Bacc("TRN2", target_bir_lowering=False, debug=False, num_devices=8)

    xt_d = nc.dram_tensor("xt", [P, HC, S], BF16, kind="ExternalInput").ap()
    wq_d = nc.dram_tensor("wq", [P, HC, H], BF16, kind="ExternalInput").ap()
    wk_d = nc.dram_tensor("wk", [P, HC, P], BF16, kind="ExternalInput").ap()
    wv_d = nc.dram_tensor("wv", [P, HC, P], BF16, kind="ExternalInput").ap()
    wo_d = nc.dram_tensor("wo", [P, HC, H], BF16, kind="ExternalInput").ap()
    bq_d = nc.dram_tensor("bqc", [P, HC], F32, kind="ExternalInput").ap()
    bk_d = nc.dram_tensor("bkc", [P, 1], F32, kind="ExternalInput").ap()
    bv_d = nc.dram_tensor("bvc", [P, 1], F32, kind="ExternalInput").ap()
    cos_d = nc.dram_tensor("cos4", [P, S], BF16, kind="ExternalInput").ap()
    sin_d = nc.dram_tensor("sinm4", [P, S], BF16, kind="ExternalInput").ap()
    pneg_d = nc.dram_tensor("pneg", [P, P], BF16, kind="ExternalInput").ap()
    ident_d = nc.dram_tensor("ident", [P, P], BF16, kind="ExternalInput").ap()
    o_d = nc.dram_tensor("o", [P, SC, H], F32, kind="ExternalOutput").ap()
    dbg = bool(int(os.environ.get("KERNEL_DEBUG", "0")))
    if dbg:
        kt_dbg = nc.dram_tensor("kt_dbg", [P, S], BF16,
                                kind="ExternalOutput").ap()
        as0_dbg = nc.dram_tensor("as0_dbg", [P, HC, QT], BF16,
                                 kind="ExternalOutput").ap()
        rcb_dbg = nc.dram_tensor("rcb_dbg", [D, 2, QT], F32,
                                 kind="ExternalOutput").ap()
        vp_dbg = nc.dram_tensor("vp_dbg", [P, SC, 130], BF16,
                                kind="ExternalOutput").ap()

    with tile.TileContext(nc) as tc, contextlib.ExitStack() as ctx:
        const = ctx.enter_context(tc.tile_pool(name="const", bufs=1))
        work = ctx.enter_context(tc.tile_pool(name="work", bufs=2))
        ppool = ctx.enter_context(tc.tile_pool(name="ppool", bufs=4))
        rpool = ctx.enter_context(tc.tile_pool(name="rpool", bufs=2))
        npool = ctx.enter_context(tc.tile_pool(name="npool", bufs=2))
        pst = ctx.enter_context(tc.tile_pool(name="pst", bufs=2, space="PSUM"))
        pav = ctx.enter_context(tc.tile_pool(name="pav", bufs=1, space="PSUM"))
        ppj = ctx.enter_context(tc.tile_pool(name="ppj", bufs=2, space="PSUM"))

        # ---- resident constants (order = DMA priority) ----
        wk_sb = const.tile([P, HC, P], BF16)
        wv_sb = const.tile([P, HC, P], BF16)
        bk_sb = const.tile([P, 1], F32)
        bv_sb = const.tile([P, 1], F32)
        bq_sb = const.tile([P, HC], F32)
        pneg_sb = const.tile([P, P], BF16)
        ident_sb = const.tile([P, P], BF16)
        wq_sb = const.tile([P, HC, H], BF16)
        wo_sb = const.tile([P, HC, H], BF16)
        bias_exp = const.tile([P, 1], F32)
        for dst, src in ((wk_sb, wk_d), (wv_sb, wv_d), (bk_sb, bk_d),
                         (bv_sb, bv_d), (bq_sb, bq_d), (pneg_sb, pneg_d),
                         (ident_sb, ident_d)):
            nc.sync.dma_start(out=dst, in_=src)
        nc.vector.memset(bias_exp, -4.0)

        # K^T resident and V' resident
        kt = const.tile([P, S], BF16)        # parts 0-63 = kv0, 64-127 = kv1
        vp = const.tile([P, SC, 130], BF16)  # [Vkv0 | ones | Vkv1 | ones]
        nc.vector.memset(vp[:, :, 64:65], 1.0)
        nc.vector.memset(vp[:, :, 129:130], 1.0)

        state = {}

        def gen_xdma(t):
            tslice = slice(t * QT, (t + 1) * QT)
            xs = work.tile([P, HC, QT], BF16, tag="xs", name=f"xs{t}")
            for c in range(HC):
                nc.sync.dma_start(out=xs[:, c, :], in_=xt_d[:, c, tslice])
            cos_t = work.tile([P, QT], BF16, tag="cos_t", name=f"cos{t}")
            sin_t = work.tile([P, QT], BF16, tag="sin_t", name=f"sin{t}")
            nc.sync.dma_start(out=cos_t, in_=cos_d[:, tslice])
            nc.sync.dma_start(out=sin_t, in_=sin_d[:, tslice])
            state[t] = {"xs": xs, "cos": cos_t, "sin": sin_t}

        # tile-0 inputs early, then the big weights
        gen_xdma(0)
        nc.sync.dma_start(out=wq_sb, in_=wq_d)
        nc.sync.dma_start(out=wo_sb, in_=wo_d)

        # absorb weight-DMA waits off the first real matmuls
        tch = ppj.tile([1, 2], F32, tag="proj", name="tch")

        def touch(t):
            ap = (t[0:1, 0, 0:2] if len(t.shape) == 3 else t[0:1, 0:2]).bitcast(F32)
            nc.tensor.matmul(tch[:, 0:1], ap, ap, start=True, stop=True)

        for t in (wk_sb, wv_sb, pneg_sb, ident_sb):
            touch(t)

        def rope_into(dst_ap, kp, bias_col, cos_t, sin_t, nm):
            """dst = (kp+b)*cos4 + Pneg @ ((kp+b)*sinm4).

            The rotate-half matmul reuses kp's psum tile (WAR on the two
            stt reads), so one proj psum buf covers the whole group.
            """
            t1 = rpool.tile([P, QT], F32, tag="t1", name=f"t1_{nm}")
            nc.vector.scalar_tensor_tensor(t1, kp, bias_col, cos_t,
                                           op0=ADD, op1=MULT)
            qe = rpool.tile([P, QT], BF16, tag="qe", name=f"qe_{nm}")
            nc.vector.scalar_tensor_tensor(qe, kp, bias_col, sin_t,
                                           op0=ADD, op1=MULT)
            yield
            nc.tensor.matmul(kp, pneg_sb, qe, start=True, stop=True)
            nc.vector.tensor_add(dst_ap, t1, kp)

        # ---------------- fill-unit generators ----------------
        def gen_kproj(t):
            st_t = state[t]
            tslice = slice(t * QT, (t + 1) * QT)
            kp = ppj.tile([P, QT], F32, tag="proj", name=f"kp{t}")
            for c in range(HC):
                nc.tensor.matmul(kp, wk_sb[:, c, :], st_t["xs"][:, c, :],
                                 start=(c == 0), stop=(c == HC - 1))
                yield
            yield from rope_into(kt[:, tslice], kp, bk_sb,
                                 st_t["cos"], st_t["sin"], f"k{t}")

        def gen_vproj(t):
            st_t = state[t]
            vtp = ppj.tile([P, QT], F32, tag="proj", name=f"vtp{t}")
            for c in range(HC):
                nc.tensor.matmul(vtp, wv_sb[:, c, :], st_t["xs"][:, c, :],
                                 start=(c == 0), stop=(c == HC - 1))
                yield
            vt_sb = rpool.tile([P, QT], BF16, tag="vt_sb", name=f"vt{t}")
            nc.vector.tensor_scalar_add(vt_sb, vtp, bv_sb)
            yield
            for j in range(4):
                sc_i = t * 4 + j
                vtr = ppj.tile([P, P], BF16, tag="proj", name=f"vtr{sc_i}")
                nc.tensor.transpose(vtr, vt_sb[:, j * P:(j + 1) * P], ident_sb)
                nc.vector.tensor_copy(vp[:, sc_i, 0:64], vtr[:, 0:64])
                nc.vector.tensor_copy(vp[:, sc_i, 65:129], vtr[:, 64:128])
                yield

        def gen_qproj(t, c):
            """Q projection chunk c (head-pair c) of tile t + rope."""
            st_t = state[t]
            qs = st_t["qs"]
            qp = ppj.tile([P, QT], F32, tag="proj", name=f"qp{t}_{c}")
            for hcc in range(HC):
                nc.tensor.matmul(qp, wq_sb[:, hcc, c * P:(c + 1) * P],
                                 st_t["xs"][:, hcc, :],
                                 start=(hcc == 0), stop=(hcc == HC - 1))
                yield
            yield from rope_into(qs[:, c, :], qp, bq_sb[:, c:c + 1],
                                 st_t["cos"], st_t["sin"], f"q{t}_{c}")

        parts = {}

        def gen_oproj_partial(j, n0, nw):
            """Last tile's o_proj: contract feat chunks 0..5 early (their
            aslab slabs are done before the final pair), park the partial
            sum in sbuf; the finisher adds chunk 6 after the last
            normalize.  Shrinks the serial epilogue tail."""
            t = NQT - 1
            aslab = state[t]["aslab"]
            sc_i = t * 4 + j
            jsl = slice(j * P, (j + 1) * P)
            op = ppj.tile([P, 512], F32, tag="proj", name=f"opp{sc_i}_{n0}")
            for c in range(HC - 1):
                nc.tensor.matmul(op[:, 0:nw], aslab[:, c, jsl],
                                 wo_sb[:, c, n0:n0 + nw],
                                 start=(c == 0), stop=(c == HC - 2))
                yield
            part = npool.tile([P, 512], F32, tag="opart", bufs=8,
                              name=f"part{sc_i}_{n0}")
            nc.scalar.copy(part[:, 0:nw], op[:, 0:nw])
            parts[(j, n0)] = part
            yield

        def oproj_finish(j, n0, nw):
            t = NQT - 1
            aslab = state[t]["aslab"]
            sc_i = t * 4 + j
            jsl = slice(j * P, (j + 1) * P)
            op = ppj.tile([P, 512], F32, tag="proj", name=f"opf{sc_i}_{n0}")
            nc.tensor.matmul(op[:, 0:nw], aslab[:, 6, jsl],
                             wo_sb[:, 6, n0:n0 + nw], start=True, stop=True)
            osb = npool.tile([P, 512], F32, tag="osb", name=f"osf{sc_i}_{n0}")
            nc.vector.tensor_add(osb[:, 0:nw], parts.pop((j, n0))[:, 0:nw],
                                 op[:, 0:nw])
            nc.sync.dma_start(out=o_d[:, sc_i, n0:n0 + nw], in_=osb[:, 0:nw])

        def gen_oproj(t, j, n0, nw):
            """o_proj group: one 128-row seq chunk x one H-half."""
            aslab = state[t]["aslab"]
            sc_i = t * 4 + j
            jsl = slice(j * P, (j + 1) * P)
            op = ppj.tile([P, 512], F32, tag="proj", name=f"op{sc_i}_{n0}")
            for c in range(HC):
                nc.tensor.matmul(op[:, 0:nw], aslab[:, c, jsl],
                                 wo_sb[:, c, n0:n0 + nw],
                                 start=(c == 0), stop=(c == HC - 1))
                yield
            osb = npool.tile([P, 512], F32, tag="osb", name=f"os{sc_i}_{n0}")
            nc.vector.tensor_copy(osb[:, 0:nw], op[:, 0:nw])
            nc.sync.dma_start(out=o_d[:, sc_i, n0:n0 + nw], in_=osb[:, 0:nw])

        # ---------------- fill scheduler (strict FIFO) ----------------
        fills = []
        debt = 0.0

        def push(g):
            fills.append(g)

        def emit_dummy():
            # keep the PE streaming when no useful fill work remains: the
            # TRN2 PE clock drops to 1.2GHz after any idle and takes 3us of
            # continuous execution to return to 2.4GHz, so a dummy 512-col
            # matmul into the spare partitions of the av psum banks is
            # cheaper than an idle.
            d = state.get("avtile")
            if d is None:
                return
            nc.tensor.matmul(d[96:128, 0, 0:QT], wq_sb[:, 0, 0:32],
                             wq_sb[:, 1, 0:QT], start=True, stop=True,
                             tile_position=(0, 96), skip_group_check=True)

        def run_fill(n, pad=False):
            while n > 0 and fills:
                try:
                    next(fills[0])
                    n -= 1
                except StopIteration:
                    fills.pop(0)
            while pad and n > 0:
                emit_dummy()
                n -= 1
            return n

        def fill_for(slack_ns):
            nonlocal debt
            debt = min(debt + slack_ns / UNIT_NS, 8.0)
            k = int(debt)
            if k > 0:
                left = run_fill(k)
                debt -= k - left

        def drain(g):
            """Emit fills (FIFO) until generator g has fully completed.

            Engine queues execute in order, so a backbone instruction that
            depends on a fill unit DEADLOCKS unless that unit is emitted
            first.  drain() enforces the deadline-critical orderings.
            """
            if g is None:
                return
            if g in opq:
                while opq:
                    gg = opq.pop(0)
                    push(gg)
                    if gg is g:
                        break
            while g in fills:
                try:
                    next(fills[0])
                except StopIteration:
                    fills.pop(0)

        # ---------------- attention backbone ----------------
        def attention_pair(t, hp):
            qs = state[t]["qs"]
            aslab = state[t]["aslab"]
            nkc = (t + 1) * 4
            av = pav.tile([P, 2, QT], F32, tag="av", name=f"av{t}_{hp}")
            state["avtile"] = av
            probs = {}

            def emit_av(kc):
                c0 = (kc - 4 * t) * P if kc >= 4 * t else 0
                csl = slice(c0, QT)
                pr = probs.pop(kc)
                nc.tensor.matmul(av[0:65, 0, csl], vp[:, kc, 0:65],
                                 pr[:, 0, csl], start=(kc == 0),
                                 stop=(kc == nkc - 1), skip_group_check=True)
                nc.tensor.matmul(av[0:65, 1, csl], vp[:, kc, 65:130],
                                 pr[:, 1, csl], start=(kc == 0),
                                 stop=(kc == nkc - 1), skip_group_check=True)

            def emit_scores(kc):
                c0 = (kc - 4 * t) * P if kc >= 4 * t else 0
                csl = slice(c0, QT)
                ksl = slice(kc * P, (kc + 1) * P)
                st = pst.tile([P, 2, QT], F32, tag="st",
                              name=f"st{t}_{hp}_{kc}")
                nc.tensor.matmul(st[:, 0, csl], kt[0:64, ksl],
                                 qs[0:64, hp, csl], start=True, stop=True)
                nc.tensor.matmul(st[:, 1, csl], kt[64:128, ksl],
                                 qs[64:128, hp, csl], start=True, stop=True,
                                 tile_position=(64, 0))
                return st

            def emit_exp(kc, st):
                c0 = (kc - 4 * t) * P if kc >= 4 * t else 0
                csl = slice(c0, QT)
                pr = ppool.tile([P, 2, QT], BF16, tag="probs",
                                name=f"pr{t}_{hp}_{kc}")
                nc.scalar.activation(pr[:, :, csl], st[:, :, csl],
                                     mybir.ActivationFunctionType.Exp,
                                     bias=bias_exp, scale=0.125)
                if kc >= 4 * t:  # diagonal chunk: mask the triangle band
                    nc.gpsimd.affine_select(
                        out=pr[:, :, c0:c0 + P],
                        in_=pr[:, :, c0:c0 + P],
                        pattern=[[0, 2], [1, P]],
                        compare_op=mybir.AluOpType.is_ge, fill=0.0,
                        base=0, channel_multiplier=-1)
                probs[kc] = pr

            # chunks processed in twos: both score pairs back-to-back (the
            # second pair's LDWEIGHTS hides under the first pair's stream),
            # then both exps, then the lagged attnV pairs.
            for g in range(nkc // 2):
                k0, k1 = 2 * g, 2 * g + 1
                st0 = emit_scores(k0)
                st1 = emit_scores(k1)
                emit_exp(k0, st0)
                emit_exp(k1, st1)
                for kc in (k0, k1):
                    if kc >= LAG:
                        emit_av(kc - LAG)
                w2 = (QT - (k0 - 4 * t) * P if k0 >= 4 * t else QT) + \
                     (QT - (k1 - 4 * t) * P if k1 >= 4 * t else QT)
                slack = 1.1 * ((2 * w2 / 1.2 + 360.0) - (3 * w2 / 2.4 + 60.0))
                fill_for(slack)
            for kc in range(max(0, nkc - LAG), nkc):
                emit_av(kc)

            # evict av psum to sbuf FAST (frees the single av buf for the
            # next pair), then normalize lazily off-psum: recip ->
            # partition_broadcast -> muls (only gate o_proj, a tile later).
            nm = f"n{t}_{hp}"
            avs = npool.tile([D, 2, QT], BF16, tag="avs", name=f"avs{nm}")
            if t == 0 and hp == HC - 1:
                nc.scalar.copy(avs, av[0:64, :, :])
            else:
                nc.vector.tensor_copy(avs, av[0:64, :, :])
            dsb = npool.tile([1, 2, QT], F32, tag="dsb", name=f"dsb{nm}")
            if t == 0 or (t == NQT - 1 and hp == HC - 1):
                # early tiles are DVE-congested and ACT-idle (same for the
                # final pair): the slow single-partition denom copy goes to
                # the scalar engine
                nc.scalar.copy(dsb, av[64:65, :, :])
            else:
                nc.vector.tensor_copy(dsb, av[64:65, :, :])
            fill_for(1500.0 if t == 0 else 800.0)
            rc1 = npool.tile([1, 2, QT], F32, tag="rc1", name=f"rc1{nm}")
            nc.vector.reciprocal_approx_fast(rc1, dsb)
            rcb = npool.tile([D, 2, QT], F32, tag="rcb", name=f"rcb{nm}")
            nc.gpsimd.partition_broadcast(rcb, rc1)
            state["rcb_last"] = rcb
            nc.vector.tensor_mul(aslab[0:64, hp, :], avs[:, 0, :],
                                 rcb[:, 0, :])
            nc.vector.tensor_mul(aslab[64:128, hp, :], avs[:, 1, :],
                                 rcb[:, 1, :])

        # ---------------- schedule ----------------
        qgen = {}
        kvgen = {}
        ogen = {}
        opq = []
        # prologue: tile-0 K/V proj and first two Q chunks, run eagerly
        state[0]["qs"] = work.tile([P, HC, QT], BF16, tag="qs", name="qs0")
        state[0]["aslab"] = work.tile([P, HC, QT], BF16, tag="aslab",
                                      bufs=3, name="aslab0")
        for g in (gen_kproj(0), gen_vproj(0), gen_qproj(0, 0), gen_qproj(0, 1)):
            for _ in g:
                pass

        for t in range(NQT):
            qn = []
            if t == 0:
                for c in range(2, HC):
                    g = gen_qproj(0, c)
                    qgen[(0, c)] = g
                    qn.append(g)
            if t + 1 < NQT:
                gen_xdma(t + 1)
                state[t + 1]["qs"] = work.tile([P, HC, QT], BF16, tag="qs",
                                               name=f"qs{t + 1}")
                state[t + 1]["aslab"] = work.tile([P, HC, QT], BF16,
                                                  tag="aslab", bufs=3,
                                                  name=f"aslab{t + 1}")
                gk, gv = gen_kproj(t + 1), gen_vproj(t + 1)
                kvgen[t + 1] = [gk, gv]
                qn.append(gk)
                qn.append(gv)
                for c in range(HC):
                    g = gen_qproj(t + 1, c)
                    qgen[(t + 1, c)] = g
                    qn.append(g)
            if t >= 1:
                ogen[t - 1] = []
                for j in range(4):
                    for n0, nw in ((0, 512), (512, 384)):
                        g = gen_oproj(t - 1, j, n0, nw)
                        ogen[t - 1].append(g)
                        opq.append(g)
            # Q/K/V have a hard deadline (next tile start) and are pushed
            # now; o_proj groups (two tiles of slack via aslab bufs=3) are
            # admitted on demand at pair starts, reserving them for the
            # fill-starved late tiles.
            for g in qn:
                push(g)

            for hp in range(HC):
                # hard ordering requirements (deadlock avoidance):
                # qs chunk written before its scores; kt/vp tile-t section
                # written before this tile's diagonal chunks; o_proj(t-2)
                # done before this tile's normalize reuses its aslab buf.
                drain(qgen.get((t, hp)))
                if hp == 0:
                    for g in kvgen.get(t, ()):
                        drain(g)
                    for g in ogen.get(t - 3, ()):
                        drain(g)
                # lookahead: finish the NEXT pair's Q group now so its rope
                # (DVE) completes before that pair's first scores issue.
                nxt = (t, hp + 1) if hp + 1 < HC else (t + 1, 0)
                drain(qgen.get(nxt))
                if t == NQT - 1 and hp == HC - 1:
                    for j in range(4):
                        for n0, nw in ((0, 512), (512, 384)):
                            push(gen_oproj_partial(j, n0, nw))
                while opq and len(fills) <= (2 if t == NQT - 1 else 1):
                    push(opq.pop(0))
                attention_pair(t, hp)

        # epilogue: drain remaining fills, then last tile's o_proj
        for g in opq:
            push(g)
        opq.clear()
        run_fill(1 << 30)
        for j in range(4):
            for n0, nw in ((0, 512), (512, 384)):
                oproj_finish(j, n0, nw)
        if dbg:
            nc.sync.dma_start(out=kt_dbg, in_=kt)
            nc.sync.dma_start(out=as0_dbg, in_=state[0]["aslab"])
            nc.sync.dma_start(out=vp_dbg, in_=vp)
            nc.sync.dma_start(out=rcb_dbg, in_=state["rcb_last"])
    nc.compile()
    return nc


def _host_prep(hidden_states, position_ids, Wq, bq, Wk, bk, Wv, bv, Wo):
    """Build per-core input maps (host-side layout work)."""
    bf16 = ml_dtypes.bfloat16
    f32 = np.float32
    HALF = 32

    def chunked(w, dt=bf16):  # [H, N] -> [P, HC, N]
        return np.ascontiguousarray(
            w.reshape(HC, P, -1).transpose(1, 0, 2)).astype(dt)

    # q head-pair permutation: chunk p partitions 0-63 = head p (kv0),
    # 64-127 = head p+7 (kv1); Wo rows use the same ordering.
    perm = np.empty(H, np.int64)
    for hp in range(HC):
        for p in range(P):
            h = hp if p < 64 else hp + 7
            perm[hp * P + p] = h * D + (p % 64)
    wq_h = chunked(Wq[:, perm])
    wk_h = chunked(Wk)
    wv_h = chunked(Wv)
    wo_h = chunked(Wo[perm])
    bq_h = np.ascontiguousarray(bq[perm].reshape(HC, P).T).astype(f32)
    bk_h = bk.reshape(P, 1).astype(f32)
    bv_h = bv.reshape(P, 1).astype(f32)
    pneg = np.zeros((P, P), f32)
    for i in range(P):
        pneg[i, i ^ 32] = -1.0
    pneg_h = pneg.astype(bf16)
    ident_h = np.eye(P, dtype=f32).astype(bf16)

    inv_freq = (1.0 / (THETA ** (np.arange(0, HALF, dtype=np.float64) / HALF)))
    pidx = np.arange(P)
    sign = np.where((pidx % 64) >= HALF, 1.0, -1.0)[:, None]

    in_maps = []
    for b in range(B):
        xt = np.ascontiguousarray(
            hidden_states[b].T.reshape(HC, P, S).transpose(1, 0, 2)).astype(bf16)
        ang = position_ids[b].astype(np.float64)[None, :] * \
            inv_freq[pidx % HALF][:, None]          # [P, S]
        cos4 = np.cos(ang).astype(bf16)
        sinm4 = (np.sin(ang) * sign).astype(bf16)
        in_maps.append({
            "xt": xt, "wq": wq_h, "wk": wk_h, "wv": wv_h, "wo": wo_h,
            "bqc": bq_h, "bkc": bk_h, "bvc": bv_h,
            "cos4": cos4, "sinm4": sinm4, "pneg": pneg_h, "ident": ident_h,
        })
    return in_maps


def kernel(**inputs):
    global LAST_RESULTS
    if "nc" not in _CACHE:
        _CACHE["nc"] = _build()
    nc = _CACHE["nc"]
    in_maps = _host_prep(**inputs)
    trace = bool(int(os.environ.get("KERNEL_TRACE", "0")))
    res = bass_utils.run_bass_kernel_spmd(
        nc, in_maps, core_ids=list(range(8)), trace=trace)
    LAST_RESULTS = res
    out = np.empty((B, S, H), np.float32)
    for b in range(B):
        o = res.results[b]["o"]              # [P, SC, H]
        out[b] = o.transpose(1, 0, 2).reshape(S, H)
    return out


# revision 29
# speedup vs baseline: 1.0122x; 1.0021x over previous
"""CosyVoice2 attention (B=8, S=2048, H=896, 14Q/2KV GQA, RoPE, causal) as a
Trainium2 Bass/Tile kernel, data-parallel over batch across 8 NeuronCores.

v3: continuous-PE schedule (499us -> ~406us).  The TRN2 PE clock ramps
0.65->1.2->2.4GHz and only reaches 2.4GHz after ~3us of gap-free
execution; any idle resets it.  v2 ping-ponged PE<->ACT per attention
chunk, so most matmuls ran at 1.2-1.5GHz (467us PE busy for ~255us of
full-clock work).  v3 keeps the PE queue saturated: the middle of the
kernel measures 95-99% PE occupancy with matmuls at full clock (216ns
per 512-col stream).

Design:
  - attention backbone per (tile t, head-pair hp): for each 128-wide k-chunk
    kc: scores pair (two K=64 matmuls on opposite PE row-halves via
    tile_position - measured concurrent, the pair costs one 512-col stream)
    -> ACT exp (scale=1/8, bias=-4 folded; bf16 out) -> attnV pair (M=65
    with denominator ones-column) LAGGED two chunks behind the exp so the
    PE never waits on ACT.
  - psum: st [128,2,512] bufs=2 (4 banks) + av [65,2,512] bufs=1 (2) +
    proj [128,512] bufs=2 (2) = exactly 8 banks.  Proj groups are strictly
    sequential (single FIFO of fill generators) so 2 bufs ping-pong
    cleanly; rope's rotate-half matmul writes back into the projection
    psum tile (WAR tracked by the tile framework).
  - all projection / o_proj / rope work is sliced into single-matmul "fill
    units" interleaved between backbone steps at a slack-proportional rate
    (ACT per chunk ~2*csl/1.2GHz vs PE backbone 3*csl/2.4GHz), so PE slack
    is filled and the clock never drops.
  - normalize per pair: reciprocal_approx_fast on the [1,1024] denom row
    (psum->sbuf), gpsimd partition_broadcast to [64,1024], two DVE muls
    writing the bf16 A^T slab.
"""

import os
import sys

for _p in ("/opt/trn_rl_repo", "/root/.axon_site/_ro/trn_rl_repo"):
    if _p not in sys.path and os.path.isdir(_p):
        sys.path.append(_p)

import contextlib

import numpy as np
import ml_dtypes

import concourse.bacc as bacc
import concourse.mybir as mybir
import concourse.tile as tile
from concourse import bass_utils

B = 8
S = 2048
H = 896
NQ = 14
NKV = 2
D = 64
THETA = 1000000.0
P = 128
HC = H // P          # 7 hidden chunks
QT = 512             # q-tile width
NQT = S // QT        # 4 q-tiles
SC = S // P          # 16 seq chunks of 128
F32 = mybir.dt.float32
BF16 = mybir.dt.bfloat16
ADD = mybir.AluOpType.add
MULT = mybir.AluOpType.mult

_CACHE = {}
LAST_RESULTS = None

LAG = 3              # attnV trails exp by LAG chunks
UNIT_NS = 216.0      # one 512-col matmul at 2.4GHz


def _build():
    nc = bacc.Bacc("TRN2", target_bir_lowering=False, debug=False, num_devices=8)

    xt_d = nc.dram_tensor("xt", [P, HC, S], BF16, kind="ExternalInput").ap()
    wq_d = nc.dram_tensor("wq", [P, HC, H], BF16, kind="ExternalInput").ap()
    wk_d = nc.dram_tensor("wk", [P, HC, P], BF16, kind="ExternalInput").ap()
    wv_d = nc.dram_tensor("wv", [P, HC, P], BF16, kind="ExternalInput").ap()
    wo_d = nc.dram_tensor("wo", [P, HC, H], BF16, kind="ExternalInput").ap()
    bq_d = nc.dram_tensor("bqc", [P, HC], F32, kind="ExternalInput").ap()
    bk_d = nc.dram_tensor("bkc", [P, 1], F32, kind="ExternalInput").ap()
    bv_d = nc.dram_tensor("bvc", [P, 1], F32, kind="ExternalInput").ap()
    cos_d = nc.dram_tensor("cos4", [P, S], BF16, kind="ExternalInput").ap()
    sin_d = nc.dram_tensor("sinm4", [P, S], BF16, kind="ExternalInput").ap()
    pneg_d = nc.dram_tensor("pneg", [P, P], BF16, kind="ExternalInput").ap()
    ident_d = nc.dram_tensor("ident", [P, P], BF16, kind="ExternalInput").ap()
    o_d = nc.dram_tensor("o", [P, SC, H], F32, kind="ExternalOutput").ap()
    dbg = bool(int(os.environ.get("KERNEL_DEBUG", "0")))
    if dbg:
        kt_dbg = nc.dram_tensor("kt_dbg", [P, S], BF16,
                                kind="ExternalOutput").ap()
        as0_dbg = nc.dram_tensor("as0_dbg", [P, HC, QT], BF16,
                                 kind="ExternalOutput").ap()
        rcb_dbg = nc.dram_tensor("rcb_dbg", [D, 2, QT], F32,
                                 kind="ExternalOutput").ap()
        vp_dbg = nc.dram_tensor("vp_dbg", [P, SC, 130], BF16,
                                kind="ExternalOutput").ap()

    with tile.TileContext(nc) as tc, contextlib.ExitStack() as ctx:
        const = ctx.enter_context(tc.tile_pool(name="const", bufs=1))
        work = ctx.enter_context(tc.tile_pool(name="work", bufs=2))
        ppool = ctx.enter_context(tc.tile_pool(name="ppool", bufs=5))
        rpool = ctx.enter_context(tc.tile_pool(name="rpool", bufs=2))
        npool = ctx.enter_context(tc.tile_pool(name="npool", bufs=2))
        pst = ctx.enter_context(tc.tile_pool(name="pst", bufs=2, space="PSUM"))
        pav = ctx.enter_context(tc.tile_pool(name="pav", bufs=1, space="PSUM"))
        ppj = ctx.enter_context(tc.tile_pool(name="ppj", bufs=2, space="PSUM"))

        # ---- resident constants (order = DMA priority) ----
        wk_sb = const.tile([P, HC, P], BF16)
        wv_sb = const.tile([P, HC, P], BF16)
        bk_sb = const.tile([P, 1], F32)
        bv_sb = const.tile([P, 1], F32)
        bq_sb = const.tile([P, HC], F32)
        pneg_sb = const.tile([P, P], BF16)
        ident_sb = const.tile([P, P], BF16)
        wq_sb = const.tile([P, HC, H], BF16)
        wo_sb = const.tile([P, HC, H], BF16)
        bias_exp = const.tile([P, 1], F32)
        for dst, src in ((wk_sb, wk_d), (wv_sb, wv_d), (bk_sb, bk_d),
                         (bv_sb, bv_d), (bq_sb, bq_d), (pneg_sb, pneg_d),
                         (ident_sb, ident_d)):
            nc.sync.dma_start(out=dst, in_=src)
        nc.vector.memset(bias_exp, -4.0)

        # K^T resident and V' resident
        kt = const.tile([P, S], BF16)        # parts 0-63 = kv0, 64-127 = kv1
        vp = const.tile([P, SC, 130], BF16)  # [Vkv0 | ones | Vkv1 | ones]
        nc.vector.memset(vp[:, :, 64:65], 1.0)
        nc.vector.memset(vp[:, :, 129:130], 1.0)

        state = {}

        def gen_xdma(t):
            tslice = slice(t * QT, (t + 1) * QT)
            xs = work.tile([P, HC, QT], BF16, tag="xs", name=f"xs{t}")
            for c in range(HC):
                nc.sync.dma_start(out=xs[:, c, :], in_=xt_d[:, c, tslice])
            cos_t = work.tile([P, QT], BF16, tag="cos_t", name=f"cos{t}")
            sin_t = work.tile([P, QT], BF16, tag="sin_t", name=f"sin{t}")
            nc.sync.dma_start(out=cos_t, in_=cos_d[:, tslice])
            nc.sync.dma_start(out=sin_t, in_=sin_d[:, tslice])
            state[t] = {"xs": xs, "cos": cos_t, "sin": sin_t}

        # tile-0 inputs early, then the big weights
        gen_xdma(0)
        nc.sync.dma_start(out=wq_sb, in_=wq_d)
        nc.sync.dma_start(out=wo_sb, in_=wo_d)

        # absorb weight-DMA waits off the first real matmuls
        tch = ppj.tile([1, 2], F32, tag="proj", name="tch")

        def touch(t):
            ap = (t[0:1, 0, 0:2] if len(t.shape) == 3 else t[0:1, 0:2]).bitcast(F32)
            nc.tensor.matmul(tch[:, 0:1], ap, ap, start=True, stop=True)

        for t in (wk_sb, wv_sb, pneg_sb, ident_sb):
            touch(t)

        def rope_into(dst_ap, kp, bias_col, cos_t, sin_t, nm):
            """dst = (kp+b)*cos4 + Pneg @ ((kp+b)*sinm4).

            The rotate-half matmul reuses kp's psum tile (WAR on the two
            stt reads), so one proj psum buf covers the whole group.
            """
            t1 = rpool.tile([P, QT], F32, tag="t1", name=f"t1_{nm}")
            nc.vector.scalar_tensor_tensor(t1, kp, bias_col, cos_t,
                                           op0=ADD, op1=MULT)
            qe = rpool.tile([P, QT], BF16, tag="qe", name=f"qe_{nm}")
            nc.vector.scalar_tensor_tensor(qe, kp, bias_col, sin_t,
                                           op0=ADD, op1=MULT)
            yield
            nc.tensor.matmul(kp, pneg_sb, qe, start=True, stop=True)
            nc.vector.tensor_add(dst_ap, t1, kp)

        # ---------------- fill-unit generators ----------------
        def gen_kproj(t):
            st_t = state[t]
            tslice = slice(t * QT, (t + 1) * QT)
            kp = ppj.tile([P, QT], F32, tag="proj", name=f"kp{t}")
            for c in range(HC):
                nc.tensor.matmul(kp, wk_sb[:, c, :], st_t["xs"][:, c, :],
                                 start=(c == 0), stop=(c == HC - 1))
                yield
            yield from rope_into(kt[:, tslice], kp, bk_sb,
                                 st_t["cos"], st_t["sin"], f"k{t}")

        def gen_vproj(t):
            st_t = state[t]
            vtp = ppj.tile([P, QT], F32, tag="proj", name=f"vtp{t}")
            for c in range(HC):
                nc.tensor.matmul(vtp, wv_sb[:, c, :], st_t["xs"][:, c, :],
                                 start=(c == 0), stop=(c == HC - 1))
                yield
            vt_sb = rpool.tile([P, QT], BF16, tag="vt_sb", name=f"vt{t}")
            nc.vector.tensor_scalar_add(vt_sb, vtp, bv_sb)
            yield
            for j in range(4):
                sc_i = t * 4 + j
                vtr = ppj.tile([P, P], BF16, tag="proj", name=f"vtr{sc_i}")
                nc.tensor.transpose(vtr, vt_sb[:, j * P:(j + 1) * P], ident_sb)
                nc.vector.tensor_copy(vp[:, sc_i, 0:64], vtr[:, 0:64])
                nc.vector.tensor_copy(vp[:, sc_i, 65:129], vtr[:, 64:128])
                yield

        def gen_qproj(t, c):
            """Q projection chunk c (head-pair c) of tile t + rope."""
            st_t = state[t]
            qs = st_t["qs"]
            qp = ppj.tile([P, QT], F32, tag="proj", name=f"qp{t}_{c}")
            for hcc in range(HC):
                nc.tensor.matmul(qp, wq_sb[:, hcc, c * P:(c + 1) * P],
                                 st_t["xs"][:, hcc, :],
                                 start=(hcc == 0), stop=(hcc == HC - 1))
                yield
            yield from rope_into(qs[:, c, :], qp, bq_sb[:, c:c + 1],
                                 st_t["cos"], st_t["sin"], f"q{t}_{c}")

        parts = {}

        def gen_oproj_partial(j, n0, nw):
            """Last tile's o_proj: contract feat chunks 0..5 early (their
            aslab slabs are done before the final pair), park the partial
            sum in sbuf; the finisher adds chunk 6 after the last
            normalize.  Shrinks the serial epilogue tail."""
            t = NQT - 1
            aslab = state[t]["aslab"]
            sc_i = t * 4 + j
            jsl = slice(j * P, (j + 1) * P)
            op = ppj.tile([P, 512], F32, tag="proj", name=f"opp{sc_i}_{n0}")
            for c in range(HC - 1):
                nc.tensor.matmul(op[:, 0:nw], aslab[:, c, jsl],
                                 wo_sb[:, c, n0:n0 + nw],
                                 start=(c == 0), stop=(c == HC - 2))
                yield
            part = npool.tile([P, 512], F32, tag="opart", bufs=8,
                              name=f"part{sc_i}_{n0}")
            nc.scalar.copy(part[:, 0:nw], op[:, 0:nw])
            parts[(j, n0)] = part
            yield

        def oproj_finish(j, n0, nw):
            t = NQT - 1
            aslab = state[t]["aslab"]
            sc_i = t * 4 + j
            jsl = slice(j * P, (j + 1) * P)
            op = ppj.tile([P, 512], F32, tag="proj", name=f"opf{sc_i}_{n0}")
            nc.tensor.matmul(op[:, 0:nw], aslab[:, 6, jsl],
                             wo_sb[:, 6, n0:n0 + nw], start=True, stop=True)
            osb = npool.tile([P, 512], F32, tag="osb", name=f"osf{sc_i}_{n0}")
            nc.vector.tensor_add(osb[:, 0:nw], parts.pop((j, n0))[:, 0:nw],
                                 op[:, 0:nw])
            nc.sync.dma_start(out=o_d[:, sc_i, n0:n0 + nw], in_=osb[:, 0:nw])

        def gen_oproj(t, j, n0, nw):
            """o_proj group: one 128-row seq chunk x one H-half."""
            aslab = state[t]["aslab"]
            sc_i = t * 4 + j
            jsl = slice(j * P, (j + 1) * P)
            op = ppj.tile([P, 512], F32, tag="proj", name=f"op{sc_i}_{n0}")
            for c in range(HC):
                nc.tensor.matmul(op[:, 0:nw], aslab[:, c, jsl],
                                 wo_sb[:, c, n0:n0 + nw],
                                 start=(c == 0), stop=(c == HC - 1))
                yield
            osb = npool.tile([P, 512], F32, tag="osb", name=f"os{sc_i}_{n0}")
            nc.vector.tensor_copy(osb[:, 0:nw], op[:, 0:nw])
            nc.sync.dma_start(out=o_d[:, sc_i, n0:n0 + nw], in_=osb[:, 0:nw])

        # ---------------- fill scheduler (strict FIFO) ----------------
        fills = []
        debt = 0.0

        def push(g):
            fills.append(g)

        def emit_dummy():
            # keep the PE streaming when no useful fill work remains: the
            # TRN2 PE clock drops to 1.2GHz after any idle and takes 3us of
            # continuous execution to return to 2.4GHz, so a dummy 512-col
            # matmul into the spare partitions of the av psum banks is
            # cheaper than an idle.
            d = state.get("avtile")
            if d is None:
                return
            nc.tensor.matmul(d[96:128, 0, 0:QT], wq_sb[:, 0, 0:32],
                             wq_sb[:, 1, 0:QT], start=True, stop=True,
                             tile_position=(0, 96), skip_group_check=True)

        def run_fill(n, pad=False):
            while n > 0 and fills:
                try:
                    next(fills[0])
                    n -= 1
                except StopIteration:
                    fills.pop(0)
            while pad and n > 0:
                emit_dummy()
                n -= 1
            return n

        def fill_for(slack_ns):
            nonlocal debt
            debt = min(debt + slack_ns / UNIT_NS, 8.0)
            k = int(debt)
            if k > 0:
                left = run_fill(k)
                debt -= k - left

        def drain(g):
            """Emit fills (FIFO) until generator g has fully completed.

            Engine queues execute in order, so a backbone instruction that
            depends on a fill unit DEADLOCKS unless that unit is emitted
            first.  drain() enforces the deadline-critical orderings.
            """
            if g is None:
                return
            if g in opq:
                while opq:
                    gg = opq.pop(0)
                    push(gg)
                    if gg is g:
                        break
            while g in fills:
                try:
                    next(fills[0])
                except StopIteration:
                    fills.pop(0)

        # ---------------- attention backbone ----------------
        def attention_pair(t, hp):
            qs = state[t]["qs"]
            aslab = state[t]["aslab"]
            nkc = (t + 1) * 4
            # deeper attnV lag on a tile's first pair: its av psum handoff
            # follows the previous tile's teardown on a congested DVE
            lag = 4 if hp == 0 else LAG
            av = pav.tile([P, 2, QT], F32, tag="av", name=f"av{t}_{hp}")
            state["avtile"] = av
            probs = {}

            def emit_av(kc):
                c0 = (kc - 4 * t) * P if kc >= 4 * t else 0
                csl = slice(c0, QT)
                pr = probs.pop(kc)
                nc.tensor.matmul(av[0:65, 0, csl], vp[:, kc, 0:65],
                                 pr[:, 0, csl], start=(kc == 0),
                                 stop=(kc == nkc - 1), skip_group_check=True)
                nc.tensor.matmul(av[0:65, 1, csl], vp[:, kc, 65:130],
                                 pr[:, 1, csl], start=(kc == 0),
                                 stop=(kc == nkc - 1), skip_group_check=True)

            def emit_scores(kc):
                c0 = (kc - 4 * t) * P if kc >= 4 * t else 0
                csl = slice(c0, QT)
                ksl = slice(kc * P, (kc + 1) * P)
                st = pst.tile([P, 2, QT], F32, tag="st",
                              name=f"st{t}_{hp}_{kc}")
                nc.tensor.matmul(st[:, 0, csl], kt[0:64, ksl],
                                 qs[0:64, hp, csl], start=True, stop=True)
                nc.tensor.matmul(st[:, 1, csl], kt[64:128, ksl],
                                 qs[64:128, hp, csl], start=True, stop=True,
                                 tile_position=(64, 0))
                return st

            def emit_exp(kc, st):
                c0 = (kc - 4 * t) * P if kc >= 4 * t else 0
                csl = slice(c0, QT)
                pr = ppool.tile([P, 2, QT], BF16, tag="probs",
                                name=f"pr{t}_{hp}_{kc}")
                nc.scalar.activation(pr[:, :, csl], st[:, :, csl],
                                     mybir.ActivationFunctionType.Exp,
                                     bias=bias_exp, scale=0.125)
                if kc >= 4 * t:  # diagonal chunk: mask the triangle band
                    nc.gpsimd.affine_select(
                        out=pr[:, :, c0:c0 + P],
                        in_=pr[:, :, c0:c0 + P],
                        pattern=[[0, 2], [1, P]],
                        compare_op=mybir.AluOpType.is_ge, fill=0.0,
                        base=0, channel_multiplier=-1)
                probs[kc] = pr

            # chunks processed in twos: both score pairs back-to-back (the
            # second pair's LDWEIGHTS hides under the first pair's stream),
            # then both exps, then the lagged attnV pairs.
            for g in range(nkc // 2):
                k0, k1 = 2 * g, 2 * g + 1
                st0 = emit_scores(k0)
                st1 = emit_scores(k1)
                emit_exp(k0, st0)
                emit_exp(k1, st1)
                for kc in (k0, k1):
                    if kc >= lag:
                        emit_av(kc - lag)
                w2 = (QT - (k0 - 4 * t) * P if k0 >= 4 * t else QT) + \
                     (QT - (k1 - 4 * t) * P if k1 >= 4 * t else QT)
                slack = 1.1 * ((2 * w2 / 1.2 + 360.0) - (3 * w2 / 2.4 + 60.0))
                fill_for(slack)
            for kc in range(max(0, nkc - lag), nkc):
                emit_av(kc)

            # evict av psum to sbuf FAST (frees the single av buf for the
            # next pair), then normalize lazily off-psum: recip ->
            # partition_broadcast -> muls (only gate o_proj, a tile later).
            nm = f"n{t}_{hp}"
            avs = npool.tile([D, 2, QT], BF16, tag="avs", name=f"avs{nm}")
            if t == 0 and hp == HC - 1:
                nc.scalar.copy(avs, av[0:64, :, :])
            else:
                nc.vector.tensor_copy(avs, av[0:64, :, :])
            dsb = npool.tile([1, 2, QT], F32, tag="dsb", name=f"dsb{nm}")
            if t == 0 or (t == NQT - 1 and hp == HC - 1):
                # early tiles are DVE-congested and ACT-idle (same for the
                # final pair): the slow single-partition denom copy goes to
                # the scalar engine
                nc.scalar.copy(dsb, av[64:65, :, :])
            else:
                nc.vector.tensor_copy(dsb, av[64:65, :, :])
            fill_for(1500.0 if t == 0 else 800.0)
            rc1 = npool.tile([1, 2, QT], F32, tag="rc1", name=f"rc1{nm}")
            nc.vector.reciprocal_approx_fast(rc1, dsb)
            rcb = npool.tile([D, 2, QT], F32, tag="rcb", name=f"rcb{nm}")
            nc.gpsimd.partition_broadcast(rcb, rc1)
            state["rcb_last"] = rcb
            nc.vector.tensor_mul(aslab[0:64, hp, :], avs[:, 0, :],
                                 rcb[:, 0, :])
            nc.vector.tensor_mul(aslab[64:128, hp, :], avs[:, 1, :],
                                 rcb[:, 1, :])

        # ---------------- schedule ----------------
        qgen = {}
        kvgen = {}
        ogen = {}
        opq = []
        # prologue: tile-0 K/V proj and first two Q chunks, run eagerly
        state[0]["qs"] = work.tile([P, HC, QT], BF16, tag="qs", name="qs0")
        state[0]["aslab"] = work.tile([P, HC, QT], BF16, tag="aslab",
                                      bufs=3, name="aslab0")
        for g in (gen_kproj(0), gen_vproj(0), gen_qproj(0, 0), gen_qproj(0, 1)):
            for _ in g:
                pass

        for t in range(NQT):
            qn = []
            if t == 0:
                for c in range(2, HC):
                    g = gen_qproj(0, c)
                    qgen[(0, c)] = g
                    qn.append(g)
            if t + 1 < NQT:
                gen_xdma(t + 1)
                state[t + 1]["qs"] = work.tile([P, HC, QT], BF16, tag="qs",
                                               name=f"qs{t + 1}")
                state[t + 1]["aslab"] = work.tile([P, HC, QT], BF16,
                                                  tag="aslab", bufs=3,
                                                  name=f"aslab{t + 1}")
                gk, gv = gen_kproj(t + 1), gen_vproj(t + 1)
                kvgen[t + 1] = [gk, gv]
                qn.append(gk)
                qn.append(gv)
                for c in range(HC):
                    g = gen_qproj(t + 1, c)
                    qgen[(t + 1, c)] = g
                    qn.append(g)
            if t >= 1:
                ogen[t - 1] = []
                for j in range(4):
                    for n0, nw in ((0, 512), (512, 384)):
                        g = gen_oproj(t - 1, j, n0, nw)
                        ogen[t - 1].append(g)
                        opq.append(g)
            # Q/K/V have a hard deadline (next tile start) and are pushed
            # now; o_proj groups (two tiles of slack via aslab bufs=3) are
            # admitted on demand at pair starts, reserving them for the
            # fill-starved late tiles.
            for g in qn:
                push(g)

            for hp in range(HC):
                # hard ordering requirements (deadlock avoidance):
                # qs chunk written before its scores; kt/vp tile-t section
                # written before this tile's diagonal chunks; o_proj(t-2)
                # done before this tile's normalize reuses its aslab buf.
                drain(qgen.get((t, hp)))
                if hp == 0:
                    for g in kvgen.get(t, ()):
                        drain(g)
                    for g in ogen.get(t - 3, ()):
                        drain(g)
                # lookahead: finish the NEXT pair's Q group now so its rope
                # (DVE) completes before that pair's first scores issue.
                nxt = (t, hp + 1) if hp + 1 < HC else (t + 1, 0)
                drain(qgen.get(nxt))
                if t == NQT - 1 and hp == HC - 1:
                    for j in range(4):
                        for n0, nw in ((0, 512), (512, 384)):
                            push(gen_oproj_partial(j, n0, nw))
                while opq and len(fills) <= (2 if t == NQT - 1 else 1):
                    push(opq.pop(0))
                attention_pair(t, hp)

        # epilogue: drain remaining fills, then last tile's o_proj
        for g in opq:
            push(g)
        opq.clear()
        run_fill(1 << 30)
        for j in range(4):
            for n0, nw in ((0, 512), (512, 384)):
                oproj_finish(j, n0, nw)
        if dbg:
            nc.sync.dma_start(out=kt_dbg, in_=kt)
            nc.sync.dma_start(out=as0_dbg, in_=state[0]["aslab"])
            nc.sync.dma_start(out=vp_dbg, in_=vp)
            nc.sync.dma_start(out=rcb_dbg, in_=state["rcb_last"])
    nc.compile()
    return nc


def _host_prep(hidden_states, position_ids, Wq, bq, Wk, bk, Wv, bv, Wo):
    """Build per-core input maps (host-side layout work)."""
    bf16 = ml_dtypes.bfloat16
    f32 = np.float32
    HALF = 32

    def chunked(w, dt=bf16):  # [H, N] -> [P, HC, N]
        return np.ascontiguousarray(
            w.reshape(HC, P, -1).transpose(1, 0, 2)).astype(dt)

    # q head-pair permutation: chunk p partitions 0-63 = head p (kv0),
    # 64-127 = head p+7 (kv1); Wo rows use the same ordering.
    perm = np.empty(H, np.int64)
    for hp in range(HC):
        for p in range(P):
            h = hp if p < 64 else hp + 7
            perm[hp * P + p] = h * D + (p % 64)
    wq_h = chunked(Wq[:, perm])
    wk_h = chunked(Wk)
    wv_h = chunked(Wv)
    wo_h = chunked(Wo[perm])
    bq_h = np.ascontiguousarray(bq[perm].reshape(HC, P).T).astype(f32)
    bk_h = bk.reshape(P, 1).astype(f32)
    bv_h = bv.reshape(P, 1).astype(f32)
    pneg = np.zeros((P, P), f32)
    for i in range(P):
        pneg[i, i ^ 32] = -1.0
    pneg_h = pneg.astype(bf16)
    ident_h = np.eye(P, dtype=f32).astype(bf16)

    inv_freq = (1.0 / (THETA ** (np.arange(0, HALF, dtype=np.float64) / HALF)))
    pidx = np.arange(P)
    sign = np.where((pidx % 64) >= HALF, 1.0, -1.0)[:, None]

    in_maps = []
    for b in range(B):
        xt = np.ascontiguousarray(
            hidden_states[b].T.reshape(HC, P, S).transpose(1, 0, 2)).astype(bf16)
        ang = position_ids[b].astype(np.float64)[None, :] * \
            inv_freq[pidx % HALF][:, None]          # [P, S]
        cos4 = np.cos(ang).astype(bf16)
        sinm4 = (np.sin(ang) * sign).astype(bf16)
        in_maps.append({
            "xt": xt, "wq": wq_h, "wk": wk_h, "wv": wv_h, "wo": wo_h,
            "bqc": bq_h, "bkc": bk_h, "bvc": bv_h,
            "cos4": cos4, "sinm4": sinm4, "pneg": pneg_h, "ident": ident_h,
        })
    return in_maps


def kernel(**inputs):
    global LAST_RESULTS
    if "nc" not in _CACHE:
        _CACHE["nc"] = _build()
    nc = _CACHE["nc"]
    in_maps = _host_prep(**inputs)
    trace = bool(int(os.environ.get("KERNEL_TRACE", "0")))
    res = bass_utils.run_bass_kernel_spmd(
        nc, in_maps, core_ids=list(range(8)), trace=trace)
    LAST_RESULTS = res
    out = np.empty((B, S, H), np.float32)
    for b in range(B):
        o = res.results[b]["o"]              # [P, SC, H]
        out[b] = o.transpose(1, 0, 2).reshape(S, H)
    return out
